# revision 1
# baseline (speedup 1.0000x reference)
"""Distributed Trainium2 (8 NeuronCore) multi-head attention kernel.

Problem: y = softmax((x Wq)(x Wk)^T * DIM**-0.5) (x Wv) Wo + bo
  x: [4096, 256], 8 heads of dim 32, scale by full-dim**-0.5 (1/16).

Sharding: head-parallel. Each core owns one head h:
  - receives x^T (replicated, [256, 4096] f32) and its head's slice of
    w_qkv/b_qkv, plus full w_out/b_out,
  - computes q^T/k^T/v^T for its head (bf16 matmuls, f32 PSUM accum),
  - flash-style attention in transposed-score layout: S^T tiles
    [128 j, 256 q] = k_j @ q_cols on the PE (wide moving operand keeps
    LDWEIGHTS rare), exp via ScalarE spline or a VectorE Schraudolph
    bf16-bit-trick (split tuned so both engines stay busy; no
    max-subtraction: |scale*s| <= ~2.6 for these inputs). P^T tiles feed
    the AV matmul directly as stationary weights with a ones-column
    appended to V producing softmax row-sums for free,
  - divides, transposes O_h -> O_h^T [32, 4096] bf16,
  - two half AllToAlls (bf16, 128KB each) overlap comm with the second
    half of compute: core j ends with all heads' rows [512j:512j+512],
  - final projection O_slice @ W_out + b_out -> out rows [512, 256] f32.

Host side only reshapes/slices/transposes inputs (data marshalling) and
concatenates the 8 row-slices of the output.
"""

import numpy as np

P = 128          # partitions
N = 4096         # sequence length
DIM = 256        # model dim
H = 8            # heads == cores
D = DIM // H     # head dim, 32
QKV = 3 * D      # 96 qkv features per head
KC = DIM // P    # 2 contraction chunks
NT = N // P      # 32 j-tiles / q-blocks
NCORES = 8
RPC = N // NCORES   # 512 output rows per core
QW = 256         # q-column width per S^T matmul (half a dest slice)
SCALE = DIM ** -0.5

# Schraudolph bf16 fast-exp: bits(exp(s*SCALE)) ~= s*FE_A + FE_B (int16)
FE_A = 128.0 * SCALE * 1.4426950408889634
FE_B = 16256.0 - 4.6

# Per-pass score-tile schedule: (is_dve, j_start, n_j). Each exp engine has
# its OWN PSUM ring (separate pools below) so the two engines' pipelines
# decouple and overlap instead of taking turns; ScalarE gets 4-j-tile groups
# (FD 1024, amortizes its 172-cycle per-instruction overhead), VectorE
# 2-j-tile groups (1 PSUM bank each).
SLOT_SCHED = ((False, 0, 4), (True, 4, 2), (False, 6, 4), (True, 10, 2),
              (False, 12, 4), (True, 16, 2), (False, 18, 4), (True, 22, 2),
              (False, 24, 4), (True, 28, 2), (True, 30, 2))
DVE_J_ROW = {}
for _d, _js, _nj in SLOT_SCHED:
    if _d:
        for _k in range(_nj):
            DVE_J_ROW[_js + _k] = len(DVE_J_ROW)

_BUILT = None


def _build():
    from contextlib import ExitStack

    import concourse.mybir as mybir
    import concourse.tile as tile
    from concourse import bacc
    from concourse.masks import make_identity

    f32 = mybir.dt.float32
    bf16 = mybir.dt.bfloat16
    i16 = mybir.dt.int16
    AF = mybir.ActivationFunctionType
    ALU = mybir.AluOpType

    nc = bacc.Bacc("TRN2", target_bir_lowering=False, debug=False,
                   num_devices=NCORES)
    xT = nc.dram_tensor("xT", [DIM, N], f32, kind="ExternalInput")
    wqkv = nc.dram_tensor("wqkv", [DIM, QKV], f32, kind="ExternalInput")
    bqkv = nc.dram_tensor("bqkv", [QKV, 1], f32, kind="ExternalInput")
    wout = nc.dram_tensor("wout", [DIM, DIM], f32, kind="ExternalInput")
    bout = nc.dram_tensor("bout", [1, DIM], f32, kind="ExternalInput")
    out = nc.dram_tensor("out", [RPC, DIM], f32, kind="ExternalOutput")

    with tile.TileContext(nc) as tc, ExitStack() as ctx:
        singles = ctx.enter_context(tc.tile_pool(name="singles", bufs=1))
        sm_pool = ctx.enter_context(tc.tile_pool(name="sm", bufs=3))
        pt_pool = ctx.enter_context(tc.tile_pool(name="ptp", bufs=2))
        # PSUM: 2 two-bank ScalarE score slots + 2 one-bank VectorE score
        # slots + 2 one-bank acc slots = 8
        st_pool = ctx.enter_context(
            tc.tile_pool(name="stp", bufs=2, space="PSUM"))
        std_pool = ctx.enter_context(
            tc.tile_pool(name="stdp", bufs=2, space="PSUM"))
        acc_pool = ctx.enter_context(
            tc.tile_pool(name="accp", bufs=2, space="PSUM"))
        dram = ctx.enter_context(
            tc.tile_pool(name="dram", bufs=1, space="DRAM"))

        # ---------------- constant / input loads ----------------
        xt32 = singles.tile([P, KC, N], f32)
        xbf = singles.tile([P, KC, N], bf16)
        for c in range(KC):
            for q4 in range(4):
                sl = slice(q4 * (N // 4), (q4 + 1) * (N // 4))
                nc.sync.dma_start(out=xt32[:, c, sl], in_=xT[c * P:(c + 1) * P, sl])
                if (c * 4 + q4) % 2 == 0:
                    nc.vector.tensor_copy(xbf[:, c, sl], xt32[:, c, sl])
                else:
                    nc.scalar.activation(xbf[:, c, sl], xt32[:, c, sl],
                                         AF.Copy)

        wq32 = singles.tile([P, KC, QKV], f32)
        for c in range(KC):
            nc.sync.dma_start(out=wq32[:, c, :], in_=wqkv[c * P:(c + 1) * P, :])
        wqbf = singles.tile([P, KC, QKV], bf16)
        nc.vector.tensor_copy(wqbf[:], wq32[:])

        bq_t = []
        for g in range(3):
            bqg = singles.tile([D, 1], f32, name=f"bq{g}", tag=f"bq{g}")
            nc.sync.dma_start(out=bqg[:], in_=bqkv[g * D:(g + 1) * D, :])
            bq_t.append(bqg)

        wo32 = singles.tile([P, KC, DIM], f32)
        for c in range(KC):
            nc.sync.dma_start(out=wo32[:, c, :], in_=wout[c * P:(c + 1) * P, :])
        wobf = singles.tile([P, KC, DIM], bf16)
        nc.vector.tensor_copy(wobf[:], wo32[:])

        bo32 = singles.tile([1, DIM], f32)
        nc.sync.dma_start(out=bo32[:], in_=bout[:, :])
        bobf = singles.tile([1, DIM], bf16)
        nc.vector.tensor_copy(bobf[:], bo32[:])

        ones1 = singles.tile([1, P], bf16)
        nc.vector.memset(ones1[:], 1.0)
        ident = singles.tile([P, P], bf16)
        make_identity(nc, ident[:])

        # ------- QKV projection -> qT/kT/vT [32, 4096] bf16 (base 0) -----
        qT = singles.tile([D, N], bf16)
        kT = singles.tile([D, N], bf16)
        vT = singles.tile([D, N], bf16)
        FT2 = 512
        for g, dst in enumerate((qT, kT, vT)):
            for t in range(N // FT2):   # 8
                ps = st_pool.tile([D, FT2], f32, tag="st")
                sl = slice(t * FT2, (t + 1) * FT2)
                for c in range(KC):
                    nc.tensor.matmul(
                        ps[:], lhsT=wqbf[:, c, g * D:(g + 1) * D],
                        rhs=xbf[:, c, sl],
                        start=(c == 0), stop=(c == KC - 1))
                osl_ = dst[:, sl]
                if (g * 8 + t) % 2 == 0:
                    nc.vector.tensor_scalar_add(osl_, ps[:], bq_t[g][:])
                else:
                    nc.scalar.activation(osl_, ps[:], AF.Identity,
                                         bias=bq_t[g][:, 0:1])

        # ---------------- v -> [128 j, 32 d] tiles (+ ones col) ----------
        vsb = singles.tile([P, NT, D + 1], bf16)
        vt = st_pool.tile([P, NT * D], bf16, tag="st")
        for j in range(NT):
            nc.tensor.transpose(vt[:, j * D:(j + 1) * D],
                                vT[:, j * P:(j + 1) * P],
                                ident[:D, :D])
        nc.vector.tensor_copy(vsb[:, :, 0:D],
                              vt.rearrange("p (j d) -> p j d", j=NT))
        nc.vector.memset(vsb[:, :, D:D + 1], 1.0)

        # ---------------- attention main loop ----------------
        # hp 0: columns [Q*512, Q*512+256) for all Q -> first half of every
        # destination core's slice; AllToAll half 0 fires after it and runs
        # under hp 1's compute.
        # O^T is kept banded: otb[k*32+d, G, r] = O(row of qb_k in pass
        # group G, feature d), with k = (pass%2)*2 + qb2 and G = pass//2.
        # Four [128, 32] epilogue outputs are staged into one [128, 128]
        # tile and transposed by the DMA xbar — no PE transposes and no
        # DVE copies in the epilogue at all.
        otb = singles.tile([P, NCORES, P], bf16)
        a2a_in = [dram.tile([NCORES, D, QW], bf16, name=f"a2ai{h_}",
                            tag=f"a2ai{h_}") for h_ in range(2)]
        a2a_out = [dram.tile([NCORES, D, QW], bf16, name=f"a2ao{h_}",
                             tag=f"a2ao{h_}") for h_ in range(2)]
        osl = singles.tile([P, KC, RPC], bf16)

        # Software-pipelined: pass p emits S^T+exp for pass p interleaved
        # with the AV+epilogue of pass p-1 (whose exps finished a pass ago,
        # so the PE never waits on the activation engines mid-stream).
        NPASS = 2 * NCORES   # 16 passes of 256 q-columns
        ptqs, ptqis = [None, None], [None, None]

        def emit_half_a2a(hp):
            # half-AllToAll: dest core c gets cols [c*512 + hp*256, +256).
            # qb (c*4 + hp*2 + x) was computed in pass pp = hp*8 + c and
            # lives in otb band k = (pp%2)*2 + x, group G = pp//2.
            for c in range(NCORES):
                pp = hp * NCORES + c
                G = pp // 2
                for x in range(2):
                    k = (pp % 2) * 2 + x
                    nc.sync.dma_start(
                        out=a2a_in[hp][c][:, x * P:(x + 1) * P],
                        in_=otb[k * D:(k + 1) * D, G, :])
            nc.gpsimd.collective_compute(
                "AllToAll", ALU.bypass,
                replica_groups=[list(range(NCORES))],
                ins=[a2a_in[hp][:].opt()], outs=[a2a_out[hp][:].opt()])
            a2a_flat = a2a_out[hp].rearrange("c d r -> (c d) r")
            for c in range(KC):
                nc.sync.dma_start(
                    out=osl[:, c, hp * QW:(hp + 1) * QW],
                    in_=a2a_flat[c * P:(c + 1) * P, :])

        def emit_half_proj(hp):
            for m2 in range(QW // P):
                mt = hp * (QW // P) + m2
                fo = acc_pool.tile([P, DIM], f32, tag="acc")
                for c in range(KC):
                    nc.tensor.matmul(
                        fo[:], lhsT=osl[:, c, mt * P:(mt + 1) * P],
                        rhs=wobf[:, c, :], start=(c == 0), stop=False)
                nc.tensor.matmul(fo[:], lhsT=ones1[:], rhs=bobf[:],
                                 start=False, stop=True)
                fout = sm_pool.tile([P, DIM], f32, tag="fout")
                nc.vector.tensor_copy(fout[:], fo[:])
                nc.sync.dma_start(out=out[mt * P:(mt + 1) * P, :],
                                  in_=fout[:])

        acc = [None, None]
        obq = [None]
        for p in range(NPASS + 1):
            if p < NPASS:
                ptqs[p % 2] = pt_pool.tile([P, NT, QW], bf16, tag="pt",
                                           name=f"ptq{p}")
                ptqis[p % 2] = pt_pool.tile([P, max(1, len(DVE_J_ROW)), QW], bf16,
                                            tag="pti", name=f"ptqi{p}")
            base = (p % NCORES) * RPC + (p // NCORES) * QW
            pbase = ((p - 1) % NCORES) * RPC + ((p - 1) // NCORES) * QW
            for slot in range(16):
                if p < NPASS and slot < len(SLOT_SCHED):
                    ptq, ptqi = ptqs[p % 2], ptqis[p % 2]
                    is_dve, js, nj = SLOT_SCHED[slot]
                    pool = std_pool if is_dve else st_pool
                    st = pool.tile([P, nj, QW], f32,
                                   tag="std" if is_dve else "st",
                                   name=f"st{p}_{slot}")
                    for jj in range(nj):
                        j = js + jj
                        nc.tensor.matmul(st[:, jj, :],
                                         lhsT=kT[:, j * P:(j + 1) * P],
                                         rhs=qT[:, base:base + QW],
                                         start=True, stop=True)
                    if is_dve:
                        # int16 bit-trick scratch, then a same-engine copy
                        # into the clean bf16 tile: the dtype bitcast never
                        # crosses an engine boundary (dep tracking around
                        # cross-engine bitcast APs proved unreliable).
                        ro = DVE_J_ROW[js]
                        sc = sm_pool.tile([P, nj, QW], i16, tag="sc",
                                          name=f"sc{p}_{slot}")
                        nc.vector.tensor_scalar(
                            sc[:], st[:],
                            scalar1=float(FE_A), scalar2=float(FE_B),
                            op0=ALU.mult, op1=ALU.add)
                        nc.vector.tensor_copy(
                            ptqi[:, ro:ro + nj, :],
                            sc[:].bitcast(bf16))
                    else:
                        nc.scalar.activation(ptq[:, js:js + nj, :],
                                             st[:], AF.Exp,
                                             scale=float(SCALE))
                if p > 0:
                    pptq, pptqi = ptqs[(p - 1) % 2], ptqis[(p - 1) % 2]
                    qb2 = slot // 8
                    if slot % 8 == 0:
                        acc[qb2] = acc_pool.tile([P, D + 1], f32, tag="acc",
                                                 name=f"acc{p}_{qb2}")
                    coff = qb2 * P
                    for j in range(4 * (slot % 8), 4 * (slot % 8) + 4):
                        if j in DVE_J_ROW:
                            src = pptqi[:, DVE_J_ROW[j], coff:coff + P]
                        else:
                            src = pptq[:, j, coff:coff + P]
                        nc.tensor.matmul(acc[qb2][:], lhsT=src,
                                         rhs=vsb[:, j, :],
                                         start=(j == 0), stop=(j == NT - 1))
                    if slot % 8 == 7:
                        pp = p - 1
                        k = (pp % 2) * 2 + qb2
                        G = pp // 2
                        if k == 0:
                            obq[0] = sm_pool.tile([P, 4, D], bf16,
                                                  tag="obq", name=f"obq{G}")
                        r = sm_pool.tile([P, 1], f32, tag="r",
                                         name=f"r{p}_{qb2}")
                        nc.vector.reciprocal(r[:], acc[qb2][:, D:D + 1])
                        nc.scalar.activation(obq[0][:, k, :],
                                             acc[qb2][:, 0:D],
                                             AF.Copy, scale=r[:, 0:1])
                        if k == 3:
                            nc.sync.dma_start_transpose(otb[:, G, :],
                                                        obq[0][:])
            if p - 1 == NCORES - 1:
                emit_half_a2a(0)
        emit_half_a2a(1)
        emit_half_proj(0)
        emit_half_proj(1)

    nc.compile()
    return nc


def _get_built():
    global _BUILT
    if _BUILT is None:
        _BUILT = _build()
    return _BUILT


def make_in_maps(x, w_qkv, b_qkv, w_out, b_out):
    x = np.asarray(x, dtype=np.float32)
    w_qkv = np.asarray(w_qkv, dtype=np.float32)
    b_qkv = np.asarray(b_qkv, dtype=np.float32)
    w_out = np.asarray(w_out, dtype=np.float32)
    b_out = np.asarray(b_out, dtype=np.float32)

    xT = np.ascontiguousarray(x.T)
    wq3 = w_qkv.reshape(DIM, 3, H, D)       # [in, (q|k|v), head, d]
    bq3 = b_qkv.reshape(3, H, D)
    in_maps = []
    for h in range(NCORES):
        in_maps.append({
            "xT": xT,
            "wqkv": np.ascontiguousarray(wq3[:, :, h, :].reshape(DIM, QKV)),
            "bqkv": np.ascontiguousarray(bq3[:, h, :].reshape(QKV, 1)),
            "wout": np.ascontiguousarray(w_out),
            "bout": np.ascontiguousarray(b_out.reshape(1, DIM)),
        })
    return in_maps


def kernel(x, w_qkv, b_qkv, w_out, b_out):
    from concourse.bass_utils import run_bass_kernel_spmd

    nc = _get_built()
    in_maps = make_in_maps(x, w_qkv, b_qkv, w_out, b_out)
    res = run_bass_kernel_spmd(nc, in_maps, core_ids=list(range(NCORES)))
    return np.concatenate([res.results[i]["out"] for i in range(NCORES)],
                          axis=0)

